# revision 1
# baseline (speedup 1.0000x reference)
"""GCN message-passing kernel for trn2 (8 NeuronCores, SPMD + AllGather).

Strategy:
  - Shard the N=100352 (padded) node dim across 8 cores (12544 rows each).
  - Each hop: every core gathers x[col] rows (fp16) for its edges via
    dma_gather, applies edge weights through a fused one-hot (is_equal*val)
    built on DVE, and segment-sums via PE matmuls accumulating in PSUM in
    transposed layout y^T [feat, dst]. Dense W matmul + bias follow, then a
    PE transpose back to row layout, written to the core's shard; an
    AllGather publishes the full x_{h} (fp16) for the next hop.
  - pos/neg pair rows for each hop are gathered (window-sorted) and
    l2-normalized on device; the host inverse-permutes into the final
    [4, 3, 50000, 128] output.
All host-side work is integer metadata packing only; all float math happens
on device (messages/one-hot in fp16, accumulation in fp32 PSUM).
"""
import os
import sys

sys.path.insert(0, "/opt/trn_rl_repo")

import numpy as np

N = 100000
D = 128
NCORES = 8
SHARD = 12544            # 98 tiles of 128
NTILE = SHARD // 128     # 98
NPAD = SHARD * NCORES    # 100352
WIN = 32768
NWIN = (NPAD + WIN - 1) // WIN  # 4
SG_TILES = 8
NSG = (NTILE + SG_TILES - 1) // SG_TILES  # 13
E_PAIR = 50000
PAIR_PER_CORE = 4 * E_PAIR // NCORES      # 25000
P = 128

_CACHE = {}
LAST_RESULTS = None  # BassKernelResults of the most recent run (for test.py)


def _ceil(a, b):
    return -(-a // b)


def _pack_idx(idx_arr, cap):
    """Pack idx list (len<=cap*128, int) to the [128, cap*8] wrapped+replicated
    int16 layout. Pads with 0 (real row-0 gathers; masked by val=0)."""
    n = cap * 128
    buf = np.zeros(n, np.int16)
    buf[: len(idx_arr)] = idx_arr.astype(np.int16)
    blk = buf.reshape(n // 16, 16).T  # [16, n/16]
    return np.tile(blk, (8, 1))       # [128, n/16]


def _prep(edge_row, edge_col, edge_val, pos_src, pos_dst, neg_src, neg_dst):
    """Build per-core metadata + the static structure description."""
    # ---- graph edges ----
    owner = edge_row // SHARD
    per_core = []
    for c in range(NCORES):
        m = owner == c
        r = edge_row[m].astype(np.int64) - c * SHARD
        col = edge_col[m].astype(np.int64)
        val = edge_val[m]
        tile = r >> 7
        slot = r & 127
        win = col >> 15
        sg = tile // SG_TILES
        order = np.lexsort((tile, win, sg))
        per_core.append(dict(
            tile=tile[order], slot=slot[order], col=col[order],
            val=val[order], win=win[order], sg=sg[order]))

    # run partitions: key = sg*NWIN + win
    run_counts = np.zeros((NCORES, NSG * NWIN), np.int64)
    run_starts = np.zeros((NCORES, NSG * NWIN + 1), np.int64)
    for c in range(NCORES):
        d = per_core[c]
        key = d["sg"] * NWIN + d["win"]
        run_counts[c] = np.bincount(key, minlength=NSG * NWIN)
        run_starts[c, 1:] = np.cumsum(run_counts[c])

    cap_blk = np.zeros(NSG * NWIN, np.int64)
    for k in range(NSG * NWIN):
        cap_blk[k] = _ceil(int(run_counts[:, k].max()), 128)

    # per-sg gather-buffer block offsets (same layout every sg; sized by max)
    sg_bof = []       # sg -> [win -> block offset within sg buffer]
    sg_nblk = []
    for s in range(NSG):
        off = [0] * NWIN
        acc = 0
        for w in range(NWIN):
            off[w] = acc
            acc += int(cap_blk[s * NWIN + w])
        sg_bof.append(off)
        sg_nblk.append(acc)
    TOTBLK = max(sg_nblk)

    # block -> union of tiles (over cores); then tile-major MM slot list per sg
    # slots: list over sg of list of (tile_local, win, blk)
    mm_slots = []
    for s in range(NSG):
        tiles_here = list(range(s * SG_TILES, min((s + 1) * SG_TILES, NTILE)))
        cover = {}
        for w in range(NWIN):
            k = s * NWIN + w
            for b in range(int(cap_blk[k])):
                u = set()
                for c in range(NCORES):
                    st = run_starts[c, k]
                    n = run_counts[c, k]
                    lo = b * 128
                    hi = min(lo + 128, n)
                    if lo < n:
                        seg = per_core[c]["tile"][st + lo: st + hi]
                        u.update(np.unique(seg).tolist())
                cover[(w, b)] = u
        slots_s = []
        for t in tiles_here:
            for w in range(NWIN):
                for b in range(int(cap_blk[s * NWIN + w])):
                    if t in cover[(w, b)]:
                        slots_s.append((t - s * SG_TILES, w, b))
        mm_slots.append(slots_s)
    NMM = sum(len(x) for x in mm_slots)

    # per-core sv (slot/val per MM slot) and gidx
    GCOLS = int(sum(cap_blk)) * 8
    gidx_arrs = []
    gsv_arrs = []
    for c in range(NCORES):
        d = per_core[c]
        gidx = np.zeros((128, GCOLS), np.int16)
        gsv = np.zeros((128, 2 * NMM), np.float32)
        gcol_off = 0
        for s in range(NSG):
            for w in range(NWIN):
                k = s * NWIN + w
                cap = int(cap_blk[k])
                if cap == 0:
                    continue
                st, n = run_starts[c, k], run_counts[c, k]
                loc = d["col"][st: st + n] - w * WIN
                gidx[:, gcol_off: gcol_off + cap * 8] = _pack_idx(loc, cap)
                gcol_off += cap * 8
        mi = 0
        for s in range(NSG):
            for (tl, w, b) in mm_slots[s]:
                k = s * NWIN + w
                st, n = run_starts[c, k], run_counts[c, k]
                lo, hi = b * 128, min(b * 128 + 128, int(n))
                scol = np.full(128, -1.0, np.float32)
                vcol = np.zeros(128, np.float32)
                if lo < n:
                    seg_t = d["tile"][st + lo: st + hi]
                    seg_s = d["slot"][st + lo: st + hi]
                    seg_v = d["val"][st + lo: st + hi]
                    sel = seg_t == (s * SG_TILES + tl)
                    scol[: hi - lo][sel] = seg_s[sel]
                    vcol[: hi - lo][sel] = seg_v[sel]
                gsv[:, 2 * mi] = scol
                gsv[:, 2 * mi + 1] = vcol
                mi += 1
        gidx_arrs.append(gidx)
        gsv_arrs.append(gsv)

    # ---- pair gathers ----
    pe_idx = np.concatenate([pos_src, pos_dst, neg_src, neg_dst]).astype(np.int64)
    pair_meta = []
    pcnts = np.zeros((NCORES, NWIN), np.int64)
    for c in range(NCORES):
        sl = pe_idx[c * PAIR_PER_CORE: (c + 1) * PAIR_PER_CORE]
        w = sl >> 15
        order = np.argsort(w, kind="stable")
        pair_meta.append((sl[order], w[order], order))
        pcnts[c] = np.bincount(w[order], minlength=NWIN)
    pcap_blk = [_ceil(int(pcnts[:, w].max()), 128) for w in range(NWIN)]
    PPAD = 128 * sum(pcap_blk)
    PCOLS = sum(pcap_blk) * 8
    pidx_arrs = []
    for c in range(NCORES):
        sidx, swin, _ = pair_meta[c]
        pidx = np.zeros((128, PCOLS), np.int16)
        off = 0
        cum = 0
        for w in range(NWIN):
            n = int(pcnts[c, w])
            cap = pcap_blk[w]
            loc = sidx[cum: cum + n] - w * WIN
            pidx[:, off: off + cap * 8] = _pack_idx(loc, cap)
            cum += n
            off += cap * 8
        pidx_arrs.append(pidx)

    structure = (
        tuple(cap_blk.tolist()),
        tuple(tuple(s) for sg in mm_slots for s in sg),
        tuple(len(s) for s in mm_slots),
        tuple(pcap_blk),
        TOTBLK,
    )
    meta = dict(
        cap_blk=cap_blk, sg_bof=sg_bof, sg_nblk=sg_nblk, TOTBLK=TOTBLK,
        mm_slots=mm_slots, NMM=NMM, GCOLS=GCOLS,
        pcap_blk=pcap_blk, PPAD=PPAD, PCOLS=PCOLS,
        gidx_arrs=gidx_arrs, gsv_arrs=gsv_arrs, pidx_arrs=pidx_arrs,
        pair_meta=pair_meta, pcnts=pcnts,
    )
    return structure, meta


def _build_program(structure, meta):
    import concourse.bass as bass
    import concourse.mybir as mybir
    import concourse.tile as tile
    from concourse import bacc
    from concourse.masks import make_identity

    f16 = mybir.dt.float16
    f32 = mybir.dt.float32
    i16 = mybir.dt.int16

    cap_blk = meta["cap_blk"]
    sg_bof = meta["sg_bof"]
    mm_slots = meta["mm_slots"]
    NMM = meta["NMM"]
    GCOLS = meta["GCOLS"]
    pcap_blk = meta["pcap_blk"]
    PPAD = meta["PPAD"]
    PCOLS = meta["PCOLS"]
    TOTBLK = meta["TOTBLK"]

    nc = bacc.Bacc(None, num_devices=NCORES)
    x0f32 = nc.dram_tensor("x0f32", [NPAD, D], f32, kind="ExternalInput")
    x0f16 = nc.dram_tensor("x0f16", [NPAD, D], f16, kind="ExternalInput")
    gidx = nc.dram_tensor("gidx", [P, GCOLS], i16, kind="ExternalInput")
    gsv = nc.dram_tensor("gsv", [P, 2 * NMM], f32, kind="ExternalInput")
    pidx = nc.dram_tensor("pidx", [P, PCOLS], i16, kind="ExternalInput")
    w1 = nc.dram_tensor("w1", [D, D], f16, kind="ExternalInput")
    w2 = nc.dram_tensor("w2", [D, D], f16, kind="ExternalInput")
    b1 = nc.dram_tensor("b1", [D, 1], f32, kind="ExternalInput")
    b2 = nc.dram_tensor("b2", [D, 1], f32, kind="ExternalInput")
    out_pairs = nc.dram_tensor("out_pairs", [3, PPAD, D], f32,
                               kind="ExternalOutput")

    with tile.TileContext(nc) as tc:
        with (
            tc.tile_pool(name="const", bufs=1) as cpool,
            tc.tile_pool(name="meta", bufs=1) as mpool,
            tc.tile_pool(name="gb", bufs=2) as gpool,
            tc.tile_pool(name="work", bufs=4) as wpool,
            tc.tile_pool(name="pw", bufs=2) as ppool,
            tc.tile_pool(name="psy", bufs=3, space="PSUM") as psy,
            tc.tile_pool(name="psx", bufs=2, space="PSUM") as psx,
            tc.tile_pool(name="psz", bufs=2, space="PSUM") as psz,
            tc.tile_pool(name="dram", bufs=1, space="DRAM") as dram,
        ):
            # constants
            ident = cpool.tile([P, P], f16)
            make_identity(nc, ident)
            iota_i = cpool.tile([P, P], mybir.dt.int32)
            nc.gpsimd.iota(iota_i, pattern=[[1, P]], base=0,
                           channel_multiplier=0)
            iota_f = cpool.tile([P, P], f32)
            nc.vector.tensor_copy(iota_f, iota_i)
            w1_t = cpool.tile([P, P], f16)
            nc.sync.dma_start(out=w1_t, in_=w1[:, :])
            w2_t = cpool.tile([P, P], f16)
            nc.sync.dma_start(out=w2_t, in_=w2[:, :])
            b1_t = cpool.tile([P, 1], f32)
            nc.sync.dma_start(out=b1_t, in_=b1[:, :])
            b2_t = cpool.tile([P, 1], f32)
            nc.sync.dma_start(out=b2_t, in_=b2[:, :])
            gidx_t = mpool.tile([P, GCOLS], i16)
            nc.sync.dma_start(out=gidx_t, in_=gidx[:, :])
            gsv_t = mpool.tile([P, 2 * NMM], f32)
            nc.sync.dma_start(out=gsv_t, in_=gsv[:, :])
            pidx_t = mpool.tile([P, PCOLS], i16)
            nc.sync.dma_start(out=pidx_t, in_=pidx[:, :])

            # internal DRAM
            xsh1 = dram.tile([SHARD, D], f16)
            xsh2 = dram.tile([SHARD, D], f16)
            xg1 = dram.tile([NPAD, D], f16, addr_space="Shared")
            xg2 = dram.tile([NPAD, D], f16, addr_space="Shared")

            def pair_stage(hop, src, is_f32):
                """Gather pair rows from src and l2norm into out_pairs[hop]."""
                sdt = f32 if is_f32 else f16
                pcol_off = 0
                row_base = 0
                for w in range(NWIN):
                    cap = pcap_blk[w]
                    hi = min(NPAD, (w + 1) * WIN)
                    src_w = src[w * WIN: hi, :]
                    for k0 in range(0, cap, 16):
                        blk = min(16, cap - k0)
                        pbuf = ppool.tile([P, 16, P], sdt, tag="pbuf")
                        nc.gpsimd.dma_gather(
                            pbuf[:, :blk, :], src_w,
                            pidx_t[:, pcol_off + k0 * 8:
                                   pcol_off + (k0 + blk) * 8],
                            num_idxs=blk * 128, num_idxs_reg=blk * 128,
                            elem_size=P, single_packet=False,
                        )
                        sq = ppool.tile([P, 16, P], f32, tag="sq")
                        nc.vector.tensor_tensor(
                            out=sq[:, :blk, :], in0=pbuf[:, :blk, :],
                            in1=pbuf[:, :blk, :], op=mybir.AluOpType.mult)
                        ss = ppool.tile([P, 16], f32, tag="ss")
                        nc.vector.tensor_reduce(
                            out=ss[:, :blk], in_=sq[:, :blk, :],
                            axis=mybir.AxisListType.X, op=mybir.AluOpType.add)
                        nrm = ppool.tile([P, 16], f32, tag="nrm")
                        nc.scalar.sqrt(nrm[:, :blk], ss[:, :blk])
                        nc.vector.tensor_scalar_max(nrm[:, :blk], nrm[:, :blk],
                                                    1e-12)
                        rinv = ppool.tile([P, 16], f32, tag="rinv")
                        nc.vector.reciprocal(rinv[:, :blk], nrm[:, :blk])
                        onrm = ppool.tile([P, 16, P], f32, tag="onrm")
                        for j in range(blk):
                            nc.scalar.mul(onrm[:, j, :], pbuf[:, j, :],
                                          rinv[:, j: j + 1])
                        dst = out_pairs[hop,
                                        row_base + k0 * 128:
                                        row_base + (k0 + blk) * 128, :]
                        nc.sync.dma_start(
                            out=dst.rearrange("(b p) d -> p b d", p=P),
                            in_=onrm[:, :blk, :])
                    pcol_off += cap * 8
                    row_base += cap * 128

            def graph_hop(src, w_t, b_t, xsh):
                """One GCN hop: x_new = A @ src @ W + b, written to xsh."""
                gcol_off = [0] * (NSG * NWIN)
                acc = 0
                for s in range(NSG):
                    for w in range(NWIN):
                        gcol_off[s * NWIN + w] = acc
                        acc += int(cap_blk[s * NWIN + w]) * 8
                mi_base = [0] * NSG
                acc = 0
                for s in range(NSG):
                    mi_base[s] = acc
                    acc += len(mm_slots[s])
                for s in range(NSG):
                    gbuf = gpool.tile([P, TOTBLK, P], f16, tag="gbuf")
                    for w in range(NWIN):
                        k = s * NWIN + w
                        cap = int(cap_blk[k])
                        if cap == 0:
                            continue
                        hi = min(NPAD, (w + 1) * WIN)
                        nc.gpsimd.dma_gather(
                            gbuf[:, sg_bof[s][w]: sg_bof[s][w] + cap, :],
                            src[w * WIN: hi, :],
                            gidx_t[:, gcol_off[k]: gcol_off[k] + cap * 8],
                            num_idxs=cap * 128, num_idxs_reg=cap * 128,
                            elem_size=P, single_packet=False,
                        )
                    # tile-major MM slots
                    slots = mm_slots[s]
                    ntiles_s = min(SG_TILES, NTILE - s * SG_TILES)
                    for t in range(ntiles_s):
                        tslots = [(i, sl) for i, sl in enumerate(slots)
                                  if sl[0] == t]
                        y_ps = psy.tile([P, P], f32, space="PSUM", tag="y")
                        for si, (i, (tl, w, b)) in enumerate(tslots):
                            m = mi_base[s] + i
                            oh = wpool.tile([P, P], f16, tag="oh")
                            nc.vector.tensor_scalar(
                                out=oh, in0=iota_f,
                                scalar1=gsv_t[:, 2 * m: 2 * m + 1],
                                scalar2=gsv_t[:, 2 * m + 1: 2 * m + 2],
                                op0=mybir.AluOpType.is_equal,
                                op1=mybir.AluOpType.mult,
                            )
                            gb = sg_bof[s][w] + b
                            nc.tensor.matmul(
                                y_ps, lhsT=gbuf[:, gb, :], rhs=oh,
                                start=(si == 0), stop=(si == len(tslots) - 1),
                            )
                        yT = wpool.tile([P, P], f16, tag="yT")
                        nc.scalar.copy(yT, y_ps)
                        x_ps = psx.tile([P, P], f32, space="PSUM", tag="x")
                        nc.tensor.matmul(x_ps, lhsT=w_t, rhs=yT,
                                         start=True, stop=True)
                        xT = wpool.tile([P, P], f16, tag="xT")
                        nc.scalar.activation(
                            xT, x_ps, mybir.ActivationFunctionType.Identity,
                            bias=b_t[:, :1])
                        z_ps = psz.tile([P, P], f16, space="PSUM", tag="z")
                        nc.tensor.transpose(z_ps, xT, ident)
                        zsb = wpool.tile([P, P], f16, tag="zsb")
                        nc.scalar.copy(zsb, z_ps)
                        gt = s * SG_TILES + t
                        nc.sync.dma_start(
                            out=xsh[gt * P: (gt + 1) * P, :], in_=zsb)

            stages = os.environ.get(
                "BASS_GNN_STAGES", "p0,h1,ag1,p1,h2,ag2,p2").split(",")
            # hop 0 pairs (exact f32 source)
            if "p0" in stages:
                pair_stage(0, x0f32, True)
            # hop 1
            if "h1" in stages:
                graph_hop(x0f16, w1_t, b1_t, xsh1)
            if "ag1" in stages:
                nc.gpsimd.collective_compute(
                    "AllGather", mybir.AluOpType.bypass,
                    replica_groups=[list(range(NCORES))],
                    ins=[xsh1.opt()], outs=[xg1.opt()],
                )
            if "p1" in stages:
                pair_stage(1, xg1, False)
            # hop 2
            if "h2" in stages:
                graph_hop(xg1, w2_t, b2_t, xsh2)
            if "ag2" in stages:
                nc.gpsimd.collective_compute(
                    "AllGather", mybir.AluOpType.bypass,
                    replica_groups=[list(range(NCORES))],
                    ins=[xsh2.opt()], outs=[xg2.opt()],
                )
            if "p2" in stages:
                pair_stage(2, xg2, False)

    nc.compile()
    return nc


def _install_ntff_shim():
    """Provide antenv.axon_hooks (missing on this image) so trace=True can
    capture NTFF profiles through the axon .so."""
    import types
    if "antenv.axon_hooks" in sys.modules:
        return
    mod = types.ModuleType("antenv.axon_hooks")
    mod._hook = None

    def set_axon_ntff_profile_hook(h):
        mod._hook = h

    def get_axon_ntff_profile_hook():
        return mod._hook

    mod.set_axon_ntff_profile_hook = set_axon_ntff_profile_hook
    mod.get_axon_ntff_profile_hook = get_axon_ntff_profile_hook
    sys.modules["antenv.axon_hooks"] = mod
    try:
        from trn_agent_boot.trn_boot import _ntff_profile_via_ctypes
        mod._hook = _ntff_profile_via_ctypes("/opt/axon/libaxon_pjrt.so")
    except Exception:
        mod._hook = None


def kernel(node_emb, attri_emb, W1, b1, W2, b2, edge_val,
           edge_row, edge_col, pos_src, pos_dst, neg_src, neg_dst):
    global LAST_RESULTS
    _install_ntff_shim()
    from concourse.bass_utils import run_bass_kernel_spmd

    structure, meta = _prep(edge_row, edge_col, edge_val,
                            pos_src, pos_dst, neg_src, neg_dst)

    import time as _time
    key = (structure, os.environ.get("BASS_GNN_STAGES", ""))
    if key in _CACHE:
        nc = _CACHE[key]
    else:
        t0 = _time.time()
        nc = _build_program(structure, meta)
        print(f"[kernel] build+schedule: {_time.time() - t0:.1f}s, "
              f"{len(nc.inst_map)} instructions", flush=True)
        _CACHE[key] = nc

    x0 = np.concatenate([node_emb, attri_emb], axis=0).astype(np.float32)
    x0p = np.zeros((NPAD, D), np.float32)
    x0p[:N] = x0
    x0p16 = x0p.astype(np.float16)

    in_maps = []
    for c in range(NCORES):
        in_maps.append({
            "x0f32": x0p,
            "x0f16": x0p16,
            "gidx": meta["gidx_arrs"][c],
            "gsv": meta["gsv_arrs"][c],
            "pidx": meta["pidx_arrs"][c],
            "w1": W1.astype(np.float16),
            "w2": W2.astype(np.float16),
            "b1": b1.reshape(D, 1).astype(np.float32),
            "b2": b2.reshape(D, 1).astype(np.float32),
        })

    trace = os.environ.get("BASS_GNN_TRACE", "0") == "1"
    t0 = _time.time()
    res = run_bass_kernel_spmd(nc, in_maps, core_ids=list(range(NCORES)),
                               trace=trace)
    print(f"[kernel] compile+run: {_time.time() - t0:.1f}s", flush=True)
    LAST_RESULTS = res

    # ---- unshard: inverse-permute pair rows ----
    out = np.zeros((4, 3, E_PAIR, D), np.float32)
    pcap_blk = meta["pcap_blk"]
    for c in range(NCORES):
        op = res.results[c]["out_pairs"]  # [3, PPAD, D]
        sidx, swin, order = meta["pair_meta"][c]
        pcnt = meta["pcnts"][c]
        # device position of sorted entry j
        wbase = np.zeros(NWIN, np.int64)
        acc = 0
        for w in range(NWIN):
            wbase[w] = acc
            acc += pcap_blk[w] * 128
        cum = np.zeros(NWIN + 1, np.int64)
        cum[1:] = np.cumsum(pcnt)
        j = np.arange(PAIR_PER_CORE)
        dev_pos = wbase[swin] + (j - cum[swin])
        # global entry ids for this core's sorted order
        g = c * PAIR_PER_CORE + order
        st = g // E_PAIR
        pi = g % E_PAIR
        for h in range(3):
            out[st, h, pi] = op[h, dev_pos]
    return out



# revision 2
# speedup vs baseline: 1.6216x; 1.6216x over previous
"""GCN message-passing kernel for trn2 (8 NeuronCores, SPMD + AllGather).

Strategy:
  - Shard the N=100352 (padded) node dim across 8 cores (12544 rows each).
  - Each hop: every core gathers x[col] rows (fp16) for its edges via
    dma_gather, applies edge weights through a fused one-hot (is_equal*val)
    built on DVE, and segment-sums via PE matmuls accumulating in PSUM in
    transposed layout y^T [feat, dst]. Dense W matmul + bias follow, then a
    PE transpose back to row layout, written to the core's shard; an
    AllGather publishes the full x_{h} (fp16) for the next hop.
  - pos/neg pair rows for each hop are gathered (window-sorted) and
    l2-normalized on device; the host inverse-permutes into the final
    [4, 3, 50000, 128] output.
All host-side work is integer metadata packing only; all float math happens
on device (messages/one-hot in fp16, accumulation in fp32 PSUM).
"""
import os
import sys

sys.path.insert(0, "/opt/trn_rl_repo")

import numpy as np

N = 100000
D = 128
NCORES = 8
SHARD = 12544            # 98 tiles of 128
NTILE = SHARD // 128     # 98
NPAD = SHARD * NCORES    # 100352
WIN = 32768
NWIN = (NPAD + WIN - 1) // WIN  # 4
SG_TILES = 8
NSG = (NTILE + SG_TILES - 1) // SG_TILES  # 13
E_PAIR = 50000
PAIR_PER_CORE = 4 * E_PAIR // NCORES      # 25000
P = 128

_CACHE = {}
LAST_RESULTS = None  # BassKernelResults of the most recent run (for test.py)


def _ceil(a, b):
    return -(-a // b)


def _pack_idx(idx_arr, cap):
    """Pack idx list (len<=cap*128, int) to the [128, cap*8] wrapped+replicated
    int16 layout. Pads with 0 (real row-0 gathers; masked by val=0)."""
    n = cap * 128
    buf = np.zeros(n, np.int16)
    buf[: len(idx_arr)] = idx_arr.astype(np.int16)
    blk = buf.reshape(n // 16, 16).T  # [16, n/16]
    return np.tile(blk, (8, 1))       # [128, n/16]


def _prep(edge_row, edge_col, edge_val, pos_src, pos_dst, neg_src, neg_dst):
    """Build per-core metadata + the static structure description."""
    # ---- graph edges ----
    owner = edge_row // SHARD
    per_core = []
    for c in range(NCORES):
        m = owner == c
        r = edge_row[m].astype(np.int64) - c * SHARD
        col = edge_col[m].astype(np.int64)
        val = edge_val[m]
        tile = r >> 7
        slot = r & 127
        win = col >> 15
        sg = tile // SG_TILES
        order = np.lexsort((tile, win, sg))
        per_core.append(dict(
            tile=tile[order], slot=slot[order], col=col[order],
            val=val[order], win=win[order], sg=sg[order]))

    # run partitions: key = sg*NWIN + win
    run_counts = np.zeros((NCORES, NSG * NWIN), np.int64)
    run_starts = np.zeros((NCORES, NSG * NWIN + 1), np.int64)
    for c in range(NCORES):
        d = per_core[c]
        key = d["sg"] * NWIN + d["win"]
        run_counts[c] = np.bincount(key, minlength=NSG * NWIN)
        run_starts[c, 1:] = np.cumsum(run_counts[c])

    cap_blk = np.zeros(NSG * NWIN, np.int64)
    for k in range(NSG * NWIN):
        cap_blk[k] = _ceil(int(run_counts[:, k].max()), 128)

    # per-sg gather-buffer block offsets (same layout every sg; sized by max)
    sg_bof = []       # sg -> [win -> block offset within sg buffer]
    sg_nblk = []
    for s in range(NSG):
        off = [0] * NWIN
        acc = 0
        for w in range(NWIN):
            off[w] = acc
            acc += int(cap_blk[s * NWIN + w])
        sg_bof.append(off)
        sg_nblk.append(acc)
    TOTBLK = max(sg_nblk)

    # block -> union of tiles (over cores); then tile-major MM slot list per sg
    # slots: list over sg of list of (tile_local, win, blk)
    mm_slots = []
    for s in range(NSG):
        tiles_here = list(range(s * SG_TILES, min((s + 1) * SG_TILES, NTILE)))
        cover = {}
        for w in range(NWIN):
            k = s * NWIN + w
            for b in range(int(cap_blk[k])):
                u = set()
                for c in range(NCORES):
                    st = run_starts[c, k]
                    n = run_counts[c, k]
                    lo = b * 128
                    hi = min(lo + 128, n)
                    if lo < n:
                        seg = per_core[c]["tile"][st + lo: st + hi]
                        u.update(np.unique(seg).tolist())
                cover[(w, b)] = u
        slots_s = []
        for t in tiles_here:
            for w in range(NWIN):
                for b in range(int(cap_blk[s * NWIN + w])):
                    if t in cover[(w, b)]:
                        slots_s.append((t - s * SG_TILES, w, b))
        mm_slots.append(slots_s)
    NMM = sum(len(x) for x in mm_slots)

    # per-core sv (slot/val per MM slot) and gidx
    GCOLS = int(sum(cap_blk)) * 8
    gidx_arrs = []
    gsv_arrs = []
    for c in range(NCORES):
        d = per_core[c]
        gidx = np.zeros((128, GCOLS), np.int16)
        gsv = np.zeros((128, 2 * NMM), np.float32)
        gcol_off = 0
        for s in range(NSG):
            for w in range(NWIN):
                k = s * NWIN + w
                cap = int(cap_blk[k])
                if cap == 0:
                    continue
                st, n = run_starts[c, k], run_counts[c, k]
                loc = d["col"][st: st + n] - w * WIN
                gidx[:, gcol_off: gcol_off + cap * 8] = _pack_idx(loc, cap)
                gcol_off += cap * 8
        mi = 0
        for s in range(NSG):
            for (tl, w, b) in mm_slots[s]:
                k = s * NWIN + w
                st, n = run_starts[c, k], run_counts[c, k]
                lo, hi = b * 128, min(b * 128 + 128, int(n))
                scol = np.full(128, -1.0, np.float32)
                vcol = np.zeros(128, np.float32)
                if lo < n:
                    seg_t = d["tile"][st + lo: st + hi]
                    seg_s = d["slot"][st + lo: st + hi]
                    seg_v = d["val"][st + lo: st + hi]
                    sel = seg_t == (s * SG_TILES + tl)
                    scol[: hi - lo][sel] = seg_s[sel]
                    vcol[: hi - lo][sel] = seg_v[sel]
                gsv[:, 2 * mi] = scol
                gsv[:, 2 * mi + 1] = vcol
                mi += 1
        gidx_arrs.append(gidx)
        gsv_arrs.append(gsv)

    # ---- pair gathers ----
    pe_idx = np.concatenate([pos_src, pos_dst, neg_src, neg_dst]).astype(np.int64)
    pair_meta = []
    pcnts = np.zeros((NCORES, NWIN), np.int64)
    for c in range(NCORES):
        sl = pe_idx[c * PAIR_PER_CORE: (c + 1) * PAIR_PER_CORE]
        w = sl >> 15
        order = np.argsort(w, kind="stable")
        pair_meta.append((sl[order], w[order], order))
        pcnts[c] = np.bincount(w[order], minlength=NWIN)
    pcap_blk = [_ceil(int(pcnts[:, w].max()), 128) for w in range(NWIN)]
    PPAD = 128 * sum(pcap_blk)
    PCOLS = sum(pcap_blk) * 8
    pidx_arrs = []
    for c in range(NCORES):
        sidx, swin, _ = pair_meta[c]
        pidx = np.zeros((128, PCOLS), np.int16)
        off = 0
        cum = 0
        for w in range(NWIN):
            n = int(pcnts[c, w])
            cap = pcap_blk[w]
            loc = sidx[cum: cum + n] - w * WIN
            pidx[:, off: off + cap * 8] = _pack_idx(loc, cap)
            cum += n
            off += cap * 8
        pidx_arrs.append(pidx)

    structure = (
        tuple(cap_blk.tolist()),
        tuple(tuple(s) for sg in mm_slots for s in sg),
        tuple(len(s) for s in mm_slots),
        tuple(pcap_blk),
        TOTBLK,
    )
    meta = dict(
        cap_blk=cap_blk, sg_bof=sg_bof, sg_nblk=sg_nblk, TOTBLK=TOTBLK,
        mm_slots=mm_slots, NMM=NMM, GCOLS=GCOLS,
        pcap_blk=pcap_blk, PPAD=PPAD, PCOLS=PCOLS,
        gidx_arrs=gidx_arrs, gsv_arrs=gsv_arrs, pidx_arrs=pidx_arrs,
        pair_meta=pair_meta, pcnts=pcnts,
    )
    return structure, meta


def _build_program(structure, meta):
    import concourse.bass as bass
    import concourse.mybir as mybir
    import concourse.tile as tile
    from concourse import bacc
    from concourse.masks import make_identity

    f16 = mybir.dt.float16
    f32 = mybir.dt.float32
    i16 = mybir.dt.int16

    cap_blk = meta["cap_blk"]
    sg_bof = meta["sg_bof"]
    mm_slots = meta["mm_slots"]
    NMM = meta["NMM"]
    GCOLS = meta["GCOLS"]
    pcap_blk = meta["pcap_blk"]
    PPAD = meta["PPAD"]
    PCOLS = meta["PCOLS"]
    TOTBLK = meta["TOTBLK"]

    nc = bacc.Bacc(None, num_devices=NCORES, num_swdge_queues=4)
    x0f32 = nc.dram_tensor("x0f32", [NPAD, D], f32, kind="ExternalInput")
    x0f16 = nc.dram_tensor("x0f16", [NPAD, D], f16, kind="ExternalInput")
    gidx = nc.dram_tensor("gidx", [P, GCOLS], i16, kind="ExternalInput")
    gsv = nc.dram_tensor("gsv", [P, 2 * NMM], f32, kind="ExternalInput")
    pidx = nc.dram_tensor("pidx", [P, PCOLS], i16, kind="ExternalInput")
    w1 = nc.dram_tensor("w1", [D, D], f16, kind="ExternalInput")
    w2 = nc.dram_tensor("w2", [D, D], f16, kind="ExternalInput")
    b1 = nc.dram_tensor("b1", [D, 1], f32, kind="ExternalInput")
    b2 = nc.dram_tensor("b2", [D, 1], f32, kind="ExternalInput")
    out_pairs = nc.dram_tensor("out_pairs", [3, PPAD, D], f32,
                               kind="ExternalOutput")

    with tile.TileContext(nc) as tc:
        with (
            tc.tile_pool(name="const", bufs=1) as cpool,
            tc.tile_pool(name="meta", bufs=1) as mpool,
            tc.tile_pool(name="gb", bufs=2) as gpool,
            tc.tile_pool(name="work", bufs=4) as wpool,
            tc.tile_pool(name="pw", bufs=2) as ppool,
            tc.tile_pool(name="psy", bufs=3, space="PSUM") as psy,
            tc.tile_pool(name="psx", bufs=2, space="PSUM") as psx,
            tc.tile_pool(name="psz", bufs=2, space="PSUM") as psz,
            tc.tile_pool(name="dram", bufs=1, space="DRAM") as dram,
        ):
            # constants
            ident = cpool.tile([P, P], f16)
            make_identity(nc, ident)
            iota_i = cpool.tile([P, P], mybir.dt.int32)
            nc.gpsimd.iota(iota_i, pattern=[[1, P]], base=0,
                           channel_multiplier=0)
            iota_f = cpool.tile([P, P], f32)
            nc.vector.tensor_copy(iota_f, iota_i)
            w1_t = cpool.tile([P, P], f16)
            nc.sync.dma_start(out=w1_t, in_=w1[:, :])
            w2_t = cpool.tile([P, P], f16)
            nc.sync.dma_start(out=w2_t, in_=w2[:, :])
            b1_t = cpool.tile([P, 1], f32)
            nc.sync.dma_start(out=b1_t, in_=b1[:, :])
            b2_t = cpool.tile([P, 1], f32)
            nc.sync.dma_start(out=b2_t, in_=b2[:, :])
            gidx_t = mpool.tile([P, GCOLS], i16)
            nc.sync.dma_start(out=gidx_t, in_=gidx[:, :])
            gsv_t = mpool.tile([P, 2 * NMM], f32)
            nc.sync.dma_start(out=gsv_t, in_=gsv[:, :])
            pidx_t = mpool.tile([P, PCOLS], i16)
            nc.sync.dma_start(out=pidx_t, in_=pidx[:, :])

            # internal DRAM
            xsh1 = dram.tile([SHARD, D], f16)
            xsh2 = dram.tile([SHARD, D], f16)
            xg1 = dram.tile([NPAD, D], f16, addr_space="Shared")
            xg2 = dram.tile([NPAD, D], f16, addr_space="Shared")

            def pair_stage(hop, src, is_f32):
                """Gather pair rows from src and l2norm into out_pairs[hop]."""
                sdt = f32 if is_f32 else f16
                pcol_off = 0
                row_base = 0
                for w in range(NWIN):
                    cap = pcap_blk[w]
                    hi = min(NPAD, (w + 1) * WIN)
                    src_w = src[w * WIN: hi, :]
                    for k0 in range(0, cap, 16):
                        blk = min(16, cap - k0)
                        pbuf = ppool.tile([P, 16, P], sdt, tag="pbuf")
                        nc.gpsimd.dma_gather(
                            pbuf[:, :blk, :], src_w,
                            pidx_t[:, pcol_off + k0 * 8:
                                   pcol_off + (k0 + blk) * 8],
                            num_idxs=blk * 128, num_idxs_reg=blk * 128,
                            elem_size=P, single_packet=False,
                            queue_num=(w * 7 + k0 // 16) % 4,
                        )
                        sq = ppool.tile([P, 16, P], f32, tag="sq")
                        nc.vector.tensor_tensor(
                            out=sq[:, :blk, :], in0=pbuf[:, :blk, :],
                            in1=pbuf[:, :blk, :], op=mybir.AluOpType.mult)
                        ss = ppool.tile([P, 16], f32, tag="ss")
                        nc.vector.tensor_reduce(
                            out=ss[:, :blk], in_=sq[:, :blk, :],
                            axis=mybir.AxisListType.X, op=mybir.AluOpType.add)
                        nrm = ppool.tile([P, 16], f32, tag="nrm")
                        nc.scalar.sqrt(nrm[:, :blk], ss[:, :blk])
                        nc.vector.tensor_scalar_max(nrm[:, :blk], nrm[:, :blk],
                                                    1e-12)
                        rinv = ppool.tile([P, 16], f32, tag="rinv")
                        nc.vector.reciprocal(rinv[:, :blk], nrm[:, :blk])
                        onrm = ppool.tile([P, 16, P], f32, tag="onrm")
                        for j in range(blk):
                            nc.scalar.mul(onrm[:, j, :], pbuf[:, j, :],
                                          rinv[:, j: j + 1])
                        dst = out_pairs[hop,
                                        row_base + k0 * 128:
                                        row_base + (k0 + blk) * 128, :]
                        nc.sync.dma_start(
                            out=dst.rearrange("(b p) d -> p b d", p=P),
                            in_=onrm[:, :blk, :])
                    pcol_off += cap * 8
                    row_base += cap * 128

            def graph_hop(src, w_t, b_t, xsh):
                """One GCN hop: x_new = A @ src @ W + b, written to xsh."""
                gcol_off = [0] * (NSG * NWIN)
                acc = 0
                for s in range(NSG):
                    for w in range(NWIN):
                        gcol_off[s * NWIN + w] = acc
                        acc += int(cap_blk[s * NWIN + w]) * 8
                mi_base = [0] * NSG
                acc = 0
                for s in range(NSG):
                    mi_base[s] = acc
                    acc += len(mm_slots[s])
                for s in range(NSG):
                    gbuf = gpool.tile([P, TOTBLK, P], f16, tag="gbuf")
                    for w in range(NWIN):
                        k = s * NWIN + w
                        cap = int(cap_blk[k])
                        if cap == 0:
                            continue
                        hi = min(NPAD, (w + 1) * WIN)
                        nc.gpsimd.dma_gather(
                            gbuf[:, sg_bof[s][w]: sg_bof[s][w] + cap, :],
                            src[w * WIN: hi, :],
                            gidx_t[:, gcol_off[k]: gcol_off[k] + cap * 8],
                            num_idxs=cap * 128, num_idxs_reg=cap * 128,
                            elem_size=P, single_packet=False,
                            queue_num=w % 4,
                        )
                    # tile-major MM slots
                    slots = mm_slots[s]
                    ntiles_s = min(SG_TILES, NTILE - s * SG_TILES)
                    for t in range(ntiles_s):
                        tslots = [(i, sl) for i, sl in enumerate(slots)
                                  if sl[0] == t]
                        y_ps = psy.tile([P, P], f32, space="PSUM", tag="y")
                        for si, (i, (tl, w, b)) in enumerate(tslots):
                            m = mi_base[s] + i
                            oh = wpool.tile([P, P], f16, tag="oh")
                            nc.vector.tensor_scalar(
                                out=oh, in0=iota_f,
                                scalar1=gsv_t[:, 2 * m: 2 * m + 1],
                                scalar2=gsv_t[:, 2 * m + 1: 2 * m + 2],
                                op0=mybir.AluOpType.is_equal,
                                op1=mybir.AluOpType.mult,
                            )
                            gb = sg_bof[s][w] + b
                            nc.tensor.matmul(
                                y_ps, lhsT=gbuf[:, gb, :], rhs=oh,
                                start=(si == 0), stop=(si == len(tslots) - 1),
                            )
                        yT = wpool.tile([P, P], f16, tag="yT")
                        nc.scalar.copy(yT, y_ps)
                        x_ps = psx.tile([P, P], f32, space="PSUM", tag="x")
                        nc.tensor.matmul(x_ps, lhsT=w_t, rhs=yT,
                                         start=True, stop=True)
                        xT = wpool.tile([P, P], f16, tag="xT")
                        nc.scalar.activation(
                            xT, x_ps, mybir.ActivationFunctionType.Identity,
                            bias=b_t[:, :1])
                        z_ps = psz.tile([P, P], f16, space="PSUM", tag="z")
                        nc.tensor.transpose(z_ps, xT, ident)
                        zsb = wpool.tile([P, P], f16, tag="zsb")
                        nc.scalar.copy(zsb, z_ps)
                        gt = s * SG_TILES + t
                        nc.sync.dma_start(
                            out=xsh[gt * P: (gt + 1) * P, :], in_=zsb)

            stages = os.environ.get(
                "BASS_GNN_STAGES", "p0,h1,ag1,p1,h2,ag2,p2").split(",")
            # hop 0 pairs (exact f32 source)
            if "p0" in stages:
                pair_stage(0, x0f32, True)
            # hop 1
            if "h1" in stages:
                graph_hop(x0f16, w1_t, b1_t, xsh1)
            if "ag1" in stages:
                nc.gpsimd.collective_compute(
                    "AllGather", mybir.AluOpType.bypass,
                    replica_groups=[list(range(NCORES))],
                    ins=[xsh1.opt()], outs=[xg1.opt()],
                )
            if "p1" in stages:
                pair_stage(1, xg1, False)
            # hop 2
            if "h2" in stages:
                graph_hop(xg1, w2_t, b2_t, xsh2)
            if "ag2" in stages:
                nc.gpsimd.collective_compute(
                    "AllGather", mybir.AluOpType.bypass,
                    replica_groups=[list(range(NCORES))],
                    ins=[xsh2.opt()], outs=[xg2.opt()],
                )
            if "p2" in stages:
                pair_stage(2, xg2, False)

    nc.compile()
    return nc


def _install_ntff_shim():
    """Provide antenv.axon_hooks (missing on this image) so trace=True can
    capture NTFF profiles through the axon .so."""
    import types
    if "antenv.axon_hooks" in sys.modules:
        return
    mod = types.ModuleType("antenv.axon_hooks")
    mod._hook = None

    def set_axon_ntff_profile_hook(h):
        mod._hook = h

    def get_axon_ntff_profile_hook():
        return mod._hook

    mod.set_axon_ntff_profile_hook = set_axon_ntff_profile_hook
    mod.get_axon_ntff_profile_hook = get_axon_ntff_profile_hook
    sys.modules["antenv.axon_hooks"] = mod
    try:
        from trn_agent_boot.trn_boot import _ntff_profile_via_ctypes
        mod._hook = _ntff_profile_via_ctypes("/opt/axon/libaxon_pjrt.so")
    except Exception:
        mod._hook = None


def kernel(node_emb, attri_emb, W1, b1, W2, b2, edge_val,
           edge_row, edge_col, pos_src, pos_dst, neg_src, neg_dst):
    global LAST_RESULTS
    _install_ntff_shim()
    from concourse.bass_utils import run_bass_kernel_spmd

    structure, meta = _prep(edge_row, edge_col, edge_val,
                            pos_src, pos_dst, neg_src, neg_dst)

    import time as _time
    key = (structure, os.environ.get("BASS_GNN_STAGES", ""))
    if key in _CACHE:
        nc = _CACHE[key]
    else:
        t0 = _time.time()
        nc = _build_program(structure, meta)
        print(f"[kernel] build+schedule: {_time.time() - t0:.1f}s, "
              f"{len(nc.inst_map)} instructions", flush=True)
        _CACHE[key] = nc

    x0 = np.concatenate([node_emb, attri_emb], axis=0).astype(np.float32)
    x0p = np.zeros((NPAD, D), np.float32)
    x0p[:N] = x0
    x0p16 = x0p.astype(np.float16)

    in_maps = []
    for c in range(NCORES):
        in_maps.append({
            "x0f32": x0p,
            "x0f16": x0p16,
            "gidx": meta["gidx_arrs"][c],
            "gsv": meta["gsv_arrs"][c],
            "pidx": meta["pidx_arrs"][c],
            "w1": W1.astype(np.float16),
            "w2": W2.astype(np.float16),
            "b1": b1.reshape(D, 1).astype(np.float32),
            "b2": b2.reshape(D, 1).astype(np.float32),
        })

    trace = os.environ.get("BASS_GNN_TRACE", "0") == "1"
    t0 = _time.time()
    res = run_bass_kernel_spmd(nc, in_maps, core_ids=list(range(NCORES)),
                               trace=trace)
    print(f"[kernel] compile+run: {_time.time() - t0:.1f}s", flush=True)
    LAST_RESULTS = res

    # ---- unshard: inverse-permute pair rows ----
    out = np.zeros((4, 3, E_PAIR, D), np.float32)
    pcap_blk = meta["pcap_blk"]
    for c in range(NCORES):
        op = res.results[c]["out_pairs"]  # [3, PPAD, D]
        sidx, swin, order = meta["pair_meta"][c]
        pcnt = meta["pcnts"][c]
        # device position of sorted entry j
        wbase = np.zeros(NWIN, np.int64)
        acc = 0
        for w in range(NWIN):
            wbase[w] = acc
            acc += pcap_blk[w] * 128
        cum = np.zeros(NWIN + 1, np.int64)
        cum[1:] = np.cumsum(pcnt)
        j = np.arange(PAIR_PER_CORE)
        dev_pos = wbase[swin] + (j - cum[swin])
        # global entry ids for this core's sorted order
        g = c * PAIR_PER_CORE + order
        st = g // E_PAIR
        pi = g % E_PAIR
        for h in range(3):
            out[st, h, pi] = op[h, dev_pos]
    return out



# revision 3
# speedup vs baseline: 2.6709x; 1.6471x over previous
"""GCN message-passing kernel for trn2 (8 NeuronCores, SPMD + AllGather).

v2 strategy:
  - Shard the N=100352 (padded) node dim across 8 cores (12544 rows each).
  - Hop h: every core gathers x[col] rows (fp16) for its edges via
    dma_gather spread over 4 SWDGE queues (parallel Q7 core pairs),
    segment-sums via PE matmuls with batched one-hot routing matrices
    (built on DVE from duplicated-pair fp16 metadata so the 16-bit 2x
    stream mode applies), then x_new = (A x) @ W + b computed row-major
    by swapping matmul operand roles (stationary=y^T, moving=W) so no
    final PE transpose is needed.
  - Pair streams: instead of gathering pair rows on device, each core
    l2-normalizes its OWN shard rows each hop and writes them out; the
    host assembles out[stream, hop, i] = xn[hop][idx[i]] (pure indexing,
    all float math on device).
  - AllGather publishes x_1 (fp16) for hop 2's gathers. No AllGather
    needed after hop 2.
"""
import os
import sys

sys.path.insert(0, "/opt/trn_rl_repo")

import numpy as np

N = 100000
D = 128
NCORES = 8
SHARD = 12544            # 98 tiles of 128
NTILE = SHARD // 128     # 98
NPAD = SHARD * NCORES    # 100352
WIN = 32768
NWIN = (NPAD + WIN - 1) // WIN  # 4
SG_TILES = 8
NSG = (NTILE + SG_TILES - 1) // SG_TILES  # 13
E_PAIR = 50000
P = 128

_CACHE = {}
LAST_RESULTS = None  # BassKernelResults of the most recent run (for test.py)


def _ceil(a, b):
    return -(-a // b)


def _pack_idx(idx_arr, cap):
    """Pack idx list (len<=cap*128, int) to the [128, cap*8] wrapped+replicated
    int16 layout. Pads with 0 (real row-0 gathers; masked by val=0)."""
    n = cap * 128
    buf = np.zeros(n, np.int16)
    buf[: len(idx_arr)] = idx_arr.astype(np.int16)
    blk = buf.reshape(n // 16, 16).T  # [16, n/16]
    return np.tile(blk, (8, 1))       # [128, n/16]


def _prep(edge_row, edge_col, edge_val):
    """Build per-core metadata + the static structure description."""
    owner = edge_row // SHARD
    per_core = []
    for c in range(NCORES):
        m = owner == c
        r = edge_row[m].astype(np.int64) - c * SHARD
        col = edge_col[m].astype(np.int64)
        val = edge_val[m]
        tile = r >> 7
        slot = r & 127
        win = col >> 15
        sg = tile // SG_TILES
        order = np.lexsort((tile, win, sg))
        per_core.append(dict(
            tile=tile[order], slot=slot[order], col=col[order],
            val=val[order], win=win[order], sg=sg[order]))

    # run partitions: key = sg*NWIN + win
    run_counts = np.zeros((NCORES, NSG * NWIN), np.int64)
    run_starts = np.zeros((NCORES, NSG * NWIN + 1), np.int64)
    for c in range(NCORES):
        d = per_core[c]
        key = d["sg"] * NWIN + d["win"]
        run_counts[c] = np.bincount(key, minlength=NSG * NWIN)
        run_starts[c, 1:] = np.cumsum(run_counts[c])

    cap_blk = np.zeros(NSG * NWIN, np.int64)
    for k in range(NSG * NWIN):
        cap_blk[k] = _ceil(int(run_counts[:, k].max()), 128)

    # per-sg gather-buffer block offsets (same layout every sg; sized by max)
    sg_bof = []
    sg_nblk = []
    for s in range(NSG):
        off = [0] * NWIN
        acc = 0
        for w in range(NWIN):
            off[w] = acc
            acc += int(cap_blk[s * NWIN + w])
        sg_bof.append(off)
        sg_nblk.append(acc)
    TOTBLK = max(sg_nblk)

    # block -> union of tiles (over cores); then tile-major MM slot list per sg
    mm_slots = []
    for s in range(NSG):
        tiles_here = list(range(s * SG_TILES, min((s + 1) * SG_TILES, NTILE)))
        cover = {}
        for w in range(NWIN):
            k = s * NWIN + w
            for b in range(int(cap_blk[k])):
                u = set()
                for c in range(NCORES):
                    st = run_starts[c, k]
                    n = run_counts[c, k]
                    lo = b * 128
                    hi = min(lo + 128, n)
                    if lo < n:
                        seg = per_core[c]["tile"][st + lo: st + hi]
                        u.update(np.unique(seg).tolist())
                cover[(w, b)] = u
        slots_s = []
        for t in tiles_here:
            for w in range(NWIN):
                for b in range(int(cap_blk[s * NWIN + w])):
                    if t in cover[(w, b)]:
                        slots_s.append((t - s * SG_TILES, w, b))
        mm_slots.append(slots_s)
    NMM = sum(len(x) for x in mm_slots)

    # per-core gidx + duplicated-pair scol2/sval2 (fp16)
    GCOLS = int(sum(cap_blk)) * 8
    gidx_arrs = []
    scol_arrs = []
    sval_arrs = []
    for c in range(NCORES):
        d = per_core[c]
        gidx = np.zeros((128, GCOLS), np.int16)
        scol2 = np.full((128, 2 * NMM), -1.0, np.float16)
        sval2 = np.zeros((128, 2 * NMM), np.float16)
        gcol_off = 0
        for s in range(NSG):
            for w in range(NWIN):
                k = s * NWIN + w
                cap = int(cap_blk[k])
                if cap == 0:
                    continue
                st, n = run_starts[c, k], run_counts[c, k]
                loc = d["col"][st: st + n] - w * WIN
                gidx[:, gcol_off: gcol_off + cap * 8] = _pack_idx(loc, cap)
                gcol_off += cap * 8
        mi = 0
        for s in range(NSG):
            for (tl, w, b) in mm_slots[s]:
                k = s * NWIN + w
                st, n = run_starts[c, k], run_counts[c, k]
                lo, hi = b * 128, min(b * 128 + 128, int(n))
                scol = np.full(128, -1.0, np.float16)
                vcol = np.zeros(128, np.float16)
                if lo < n:
                    seg_t = d["tile"][st + lo: st + hi]
                    seg_s = d["slot"][st + lo: st + hi]
                    seg_v = d["val"][st + lo: st + hi]
                    sel = seg_t == (s * SG_TILES + tl)
                    scol[: hi - lo][sel] = seg_s[sel]
                    vcol[: hi - lo][sel] = seg_v[sel].astype(np.float16)
                scol2[:, 2 * mi] = scol
                scol2[:, 2 * mi + 1] = scol
                sval2[:, 2 * mi] = vcol
                sval2[:, 2 * mi + 1] = vcol
                mi += 1
        gidx_arrs.append(gidx)
        scol_arrs.append(scol2)
        sval_arrs.append(sval2)

    structure = (
        tuple(cap_blk.tolist()),
        tuple(tuple(sl) for sg in mm_slots for sl in sg),
        tuple(len(sl) for sl in mm_slots),
        TOTBLK,
    )
    meta = dict(
        cap_blk=cap_blk, sg_bof=sg_bof, sg_nblk=sg_nblk, TOTBLK=TOTBLK,
        mm_slots=mm_slots, NMM=NMM, GCOLS=GCOLS,
        gidx_arrs=gidx_arrs, scol_arrs=scol_arrs, sval_arrs=sval_arrs,
    )
    return structure, meta


def _build_program(structure, meta):
    import concourse.bass as bass
    import concourse.mybir as mybir
    import concourse.tile as tile
    from concourse import bacc

    f16 = mybir.dt.float16
    f32 = mybir.dt.float32
    i16 = mybir.dt.int16
    AP = bass.AP

    cap_blk = meta["cap_blk"]
    sg_bof = meta["sg_bof"]
    mm_slots = meta["mm_slots"]
    NMM = meta["NMM"]
    GCOLS = meta["GCOLS"]
    TOTBLK = meta["TOTBLK"]

    nc = bacc.Bacc(None, num_devices=NCORES, num_swdge_queues=4)
    x0sh = nc.dram_tensor("x0sh", [SHARD, D], f32, kind="ExternalInput")
    x0f16 = nc.dram_tensor("x0f16", [NPAD, D], f16, kind="ExternalInput")
    gidx = nc.dram_tensor("gidx", [P, GCOLS], i16, kind="ExternalInput")
    scol2 = nc.dram_tensor("scol2", [P, 2 * NMM], f16, kind="ExternalInput")
    sval2 = nc.dram_tensor("sval2", [P, 2 * NMM], f16, kind="ExternalInput")
    w1 = nc.dram_tensor("w1", [D, D], f16, kind="ExternalInput")
    w2 = nc.dram_tensor("w2", [D, D], f16, kind="ExternalInput")
    brep1 = nc.dram_tensor("brep1", [P, D], f32, kind="ExternalInput")
    brep2 = nc.dram_tensor("brep2", [P, D], f32, kind="ExternalInput")
    xn_out = nc.dram_tensor("xn", [3, SHARD, D], f32, kind="ExternalOutput")

    # mm slot base offset per sg
    mi_base = [0] * NSG
    acc = 0
    for s in range(NSG):
        mi_base[s] = acc
        acc += len(mm_slots[s])
    # gidx col offsets per (s, w)
    gcol_off = [0] * (NSG * NWIN)
    acc = 0
    for s in range(NSG):
        for w in range(NWIN):
            gcol_off[s * NWIN + w] = acc
            acc += int(cap_blk[s * NWIN + w]) * 8

    with tile.TileContext(nc) as tc:
        with (
            tc.tile_pool(name="const", bufs=1) as cpool,
            tc.tile_pool(name="meta", bufs=1) as mpool,
            tc.tile_pool(name="gb", bufs=2) as gpool,
            tc.tile_pool(name="strip", bufs=3) as spool,
            tc.tile_pool(name="work", bufs=4) as wpool,
            tc.tile_pool(name="norm", bufs=2) as npool,
            tc.tile_pool(name="psy", bufs=4, space="PSUM") as psy,
            tc.tile_pool(name="psx", bufs=2, space="PSUM") as psx,
            tc.tile_pool(name="dram", bufs=1, space="DRAM") as dram,
        ):
            # constants
            iota_i = cpool.tile([P, P], mybir.dt.int32)
            nc.gpsimd.iota(iota_i, pattern=[[1, P]], base=0,
                           channel_multiplier=0)
            iota16 = cpool.tile([P, P], f16)
            nc.vector.tensor_copy(iota16, iota_i)
            w1_t = cpool.tile([P, P], f16)
            nc.sync.dma_start(out=w1_t, in_=w1[:, :])
            w2_t = cpool.tile([P, P], f16)
            nc.sync.dma_start(out=w2_t, in_=w2[:, :])
            b1_t = cpool.tile([P, P], f32)
            nc.sync.dma_start(out=b1_t, in_=brep1[:, :])
            b2_t = cpool.tile([P, P], f32)
            nc.sync.dma_start(out=b2_t, in_=brep2[:, :])
            gidx_t = mpool.tile([P, GCOLS], i16)
            nc.sync.dma_start(out=gidx_t, in_=gidx[:, :])
            scol_t = mpool.tile([P, 2 * NMM], f16)
            nc.sync.dma_start(out=scol_t, in_=scol2[:, :])
            sval_t = mpool.tile([P, 2 * NMM], f16)
            nc.sync.dma_start(out=sval_t, in_=sval2[:, :])

            # internal DRAM
            xsh1 = dram.tile([SHARD, D], f16)
            xg1 = dram.tile([NPAD, D], f16, addr_space="Shared")

            def strip_aps(strip, strip2, m0, S):
                """4-D packed-pair APs for the batched one-hot build."""
                st = strip[:, :, :]
                st4 = AP(st.tensor, st.offset,
                         [st.ap[0], [128, S], [2, 64], [1, 2]])
                st2 = strip2[:, :, :]
                st24 = AP(st2.tensor, st2.offset,
                          [st2.ap[0], [128, S], [2, 64], [1, 2]])
                io = iota16[:, :]
                io4 = AP(io.tensor, io.offset,
                         [io.ap[0], [0, S], [2, 64], [1, 2]])
                sc = scol_t[:, 2 * m0: 2 * (m0 + S)]
                sc4 = AP(sc.tensor, sc.offset,
                         [sc.ap[0], [2, S], [0, 64], [1, 2]])
                sv = sval_t[:, 2 * m0: 2 * (m0 + S)]
                sv4 = AP(sv.tensor, sv.offset,
                         [sv.ap[0], [2, S], [0, 64], [1, 2]])
                return st4, st24, io4, sc4, sv4

            def norm_rows(xin, ntl, dst_ap):
                """l2-normalize rows of xin [P, ntl, P] -> dst (f32)."""
                sq = npool.tile([P, SG_TILES, P], f32, tag="sq")
                nc.vector.tensor_tensor(
                    out=sq[:, :ntl, :], in0=xin[:, :ntl, :],
                    in1=xin[:, :ntl, :], op=mybir.AluOpType.mult)
                rs = npool.tile([P, SG_TILES], f32, tag="rs")
                nc.vector.tensor_reduce(
                    out=rs[:, :ntl], in_=sq[:, :ntl, :],
                    axis=mybir.AxisListType.X, op=mybir.AluOpType.add)
                nrm = npool.tile([P, SG_TILES], f32, tag="nrm")
                nc.scalar.sqrt(nrm[:, :ntl], rs[:, :ntl])
                nc.vector.tensor_scalar_max(nrm[:, :ntl], nrm[:, :ntl], 1e-12)
                rinv = npool.tile([P, SG_TILES], f32, tag="rinv")
                nc.vector.reciprocal(rinv[:, :ntl], nrm[:, :ntl])
                xo = npool.tile([P, SG_TILES, P], f32, tag="xo")
                ri = rinv[:, :ntl]
                ri_b = AP(ri.tensor, ri.offset, [ri.ap[0], [1, ntl], [0, P]])
                nc.vector.tensor_tensor(
                    out=xo[:, :ntl, :], in0=xin[:, :ntl, :], in1=ri_b,
                    op=mybir.AluOpType.mult)
                nc.sync.dma_start(
                    out=dst_ap.rearrange("(c p) d -> p c d", p=P),
                    in_=xo[:, :ntl, :])

            def pair0_stage():
                """hop-0: l2norm own shard rows from exact f32 input."""
                for s in range(NSG):
                    ntl = min(SG_TILES, NTILE - s * SG_TILES)
                    r0 = s * SG_TILES * 128
                    x0t = npool.tile([P, SG_TILES, P], f32, tag="x0t")
                    nc.sync.dma_start(
                        out=x0t[:, :ntl, :],
                        in_=x0sh[r0: r0 + ntl * 128, :].rearrange(
                            "(c p) d -> p c d", p=P))
                    norm_rows(x0t, ntl, xn_out[0, r0: r0 + ntl * 128, :])

            def graph_hop(src, w_t, b_t, xsh, hop):
                """One GCN hop: x_new = (A @ src) @ W + b; write l2norm of
                own-shard rows to xn_out[hop]; optionally publish xsh."""
                for s in range(NSG):
                    gbuf = gpool.tile([P, TOTBLK, P], f16, tag="gbuf")
                    for w in range(NWIN):
                        k = s * NWIN + w
                        cap = int(cap_blk[k])
                        if cap == 0:
                            continue
                        hi = min(NPAD, (w + 1) * WIN)
                        nc.gpsimd.dma_gather(
                            gbuf[:, sg_bof[s][w]: sg_bof[s][w] + cap, :],
                            src[w * WIN: hi, :],
                            gidx_t[:, gcol_off[k]: gcol_off[k] + cap * 8],
                            num_idxs=cap * 128, num_idxs_reg=cap * 128,
                            elem_size=P, single_packet=False,
                            queue_num=w % 4,
                        )
                    slots = mm_slots[s]
                    ntl = min(SG_TILES, NTILE - s * SG_TILES)
                    xrows = npool.tile([P, SG_TILES, P], f16, tag="xrows")
                    for t in range(ntl):
                        tslots = [(i, sl) for i, sl in enumerate(slots)
                                  if sl[0] == t]
                        S = len(tslots)
                        m0 = mi_base[s] + tslots[0][0]
                        strip = spool.tile([P, S, P], f16, tag="strip")
                        strip2 = spool.tile([P, S, P], f16, tag="strip2")
                        st4, st24, io4, sc4, sv4 = strip_aps(
                            strip, strip2, m0, S)
                        nc.vector.tensor_tensor(
                            out=st4, in0=io4, in1=sc4,
                            op=mybir.AluOpType.is_equal)
                        nc.vector.tensor_tensor(
                            out=st24, in0=st4, in1=sv4,
                            op=mybir.AluOpType.mult)
                        y_ps = psy.tile([P, P], f32, space="PSUM", tag="y")
                        for si, (i, (tl, w, b)) in enumerate(tslots):
                            gb = sg_bof[s][w] + b
                            nc.tensor.matmul(
                                y_ps, lhsT=gbuf[:, gb, :],
                                rhs=strip2[:, si, :],
                                start=(si == 0), stop=(si == S - 1),
                            )
                        yT = wpool.tile([P, P], f16, tag="yT")
                        nc.scalar.copy(yT, y_ps)
                        x_ps = psx.tile([P, P], f32, space="PSUM", tag="x")
                        nc.tensor.matmul(x_ps, lhsT=yT, rhs=w_t,
                                         start=True, stop=True)
                        nc.vector.tensor_tensor(
                            out=xrows[:, t, :], in0=x_ps[:, :], in1=b_t[:, :],
                            op=mybir.AluOpType.add)
                    r0 = s * SG_TILES * 128
                    if xsh is not None:
                        nc.sync.dma_start(
                            out=xsh[r0: r0 + ntl * 128, :].rearrange(
                                "(c p) d -> p c d", p=P),
                            in_=xrows[:, :ntl, :])
                    norm_rows(xrows, ntl, xn_out[hop, r0: r0 + ntl * 128, :])

            stages = os.environ.get(
                "BASS_GNN_STAGES", "p0,h1,ag1,h2").split(",")
            if "p0" in stages:
                pair0_stage()
            if "h1" in stages:
                graph_hop(x0f16, w1_t, b1_t, xsh1, 1)
            if "ag1" in stages:
                nc.gpsimd.collective_compute(
                    "AllGather", mybir.AluOpType.bypass,
                    replica_groups=[list(range(NCORES))],
                    ins=[xsh1.opt()], outs=[xg1.opt()],
                )
            if "h2" in stages:
                graph_hop(xg1, w2_t, b2_t, None, 2)

    nc.compile()
    return nc


def _install_ntff_shim():
    """Provide antenv.axon_hooks (missing on this image) so trace=True can
    capture NTFF profiles through the axon .so."""
    import types
    if "antenv.axon_hooks" in sys.modules:
        return
    mod = types.ModuleType("antenv.axon_hooks")
    mod._hook = None

    def set_axon_ntff_profile_hook(h):
        mod._hook = h

    def get_axon_ntff_profile_hook():
        return mod._hook

    mod.set_axon_ntff_profile_hook = set_axon_ntff_profile_hook
    mod.get_axon_ntff_profile_hook = get_axon_ntff_profile_hook
    sys.modules["antenv.axon_hooks"] = mod
    try:
        from trn_agent_boot.trn_boot import _ntff_profile_via_ctypes
        mod._hook = _ntff_profile_via_ctypes("/opt/axon/libaxon_pjrt.so")
    except Exception:
        mod._hook = None


def kernel(node_emb, attri_emb, W1, b1, W2, b2, edge_val,
           edge_row, edge_col, pos_src, pos_dst, neg_src, neg_dst):
    global LAST_RESULTS
    _install_ntff_shim()
    from concourse.bass_utils import run_bass_kernel_spmd

    structure, meta = _prep(edge_row, edge_col, edge_val)

    import time as _time
    key = (structure, os.environ.get("BASS_GNN_STAGES", ""))
    if key in _CACHE:
        nc = _CACHE[key]
    else:
        t0 = _time.time()
        nc = _build_program(structure, meta)
        print(f"[kernel] build+schedule: {_time.time() - t0:.1f}s, "
              f"{len(nc.inst_map)} instructions", flush=True)
        _CACHE[key] = nc

    x0 = np.concatenate([node_emb, attri_emb], axis=0).astype(np.float32)
    x0p = np.zeros((NPAD, D), np.float32)
    x0p[:N] = x0
    x0p16 = x0p.astype(np.float16)

    in_maps = []
    for c in range(NCORES):
        in_maps.append({
            "x0sh": x0p[c * SHARD: (c + 1) * SHARD],
            "x0f16": x0p16,
            "gidx": meta["gidx_arrs"][c],
            "scol2": meta["scol_arrs"][c],
            "sval2": meta["sval_arrs"][c],
            "w1": W1.astype(np.float16),
            "w2": W2.astype(np.float16),
            "brep1": np.broadcast_to(
                b1.astype(np.float32)[None, :], (P, D)).copy(),
            "brep2": np.broadcast_to(
                b2.astype(np.float32)[None, :], (P, D)).copy(),
        })

    trace = os.environ.get("BASS_GNN_TRACE", "0") == "1"
    t0 = _time.time()
    res = run_bass_kernel_spmd(nc, in_maps, core_ids=list(range(NCORES)),
                               trace=trace)
    print(f"[kernel] compile+run: {_time.time() - t0:.1f}s", flush=True)
    LAST_RESULTS = res

    # ---- host assembly: index normalized tables per hop ----
    xn_full = np.empty((3, NPAD, D), np.float32)
    for c in range(NCORES):
        xn_full[:, c * SHARD: (c + 1) * SHARD] = res.results[c]["xn"]
    out = np.empty((4, 3, E_PAIR, D), np.float32)
    for st, idx in enumerate((pos_src, pos_dst, neg_src, neg_dst)):
        idx64 = idx.astype(np.int64)
        for h in range(3):
            out[st, h] = xn_full[h, idx64]
    return out


# revision 8
# speedup vs baseline: 3.7732x; 1.4127x over previous
"""GCN message-passing kernel for trn2 (8 NeuronCores, SPMD + AllGather).

v2 strategy:
  - Shard the N=100352 (padded) node dim across 8 cores (12544 rows each).
  - Hop h: every core gathers x[col] rows (fp16) for its edges via
    dma_gather spread over 4 SWDGE queues (parallel Q7 core pairs),
    segment-sums via PE matmuls with batched one-hot routing matrices
    (built on DVE from duplicated-pair fp16 metadata so the 16-bit 2x
    stream mode applies), then x_new = (A x) @ W + b computed row-major
    by swapping matmul operand roles (stationary=y^T, moving=W) so no
    final PE transpose is needed.
  - Pair streams: instead of gathering pair rows on device, each core
    l2-normalizes its OWN shard rows each hop and writes them out; the
    host assembles out[stream, hop, i] = xn[hop][idx[i]] (pure indexing,
    all float math on device).
  - AllGather publishes x_1 (fp16) for hop 2's gathers. No AllGather
    needed after hop 2.
"""
import os
import sys

sys.path.insert(0, "/opt/trn_rl_repo")

import numpy as np

N = 100000
D = 128
NCORES = 8
SHARD = 12544            # 98 tiles of 128
NTILE = SHARD // 128     # 98
NPAD = SHARD * NCORES    # 100352
WIN = 32768
NWIN = (NPAD + WIN - 1) // WIN  # 4
SG_TILES = 8
NSG = (NTILE + SG_TILES - 1) // SG_TILES  # 13
E_PAIR = 50000
P = 128

_CACHE = {}
LAST_RESULTS = None  # BassKernelResults of the most recent run (for test.py)


def _ceil(a, b):
    return -(-a // b)


def _pack_idx(idx_arr, cap):
    """Pack idx list (len<=cap*128, int) to the [128, cap*8] wrapped+replicated
    int16 layout. Pads with 0 (real row-0 gathers; masked by val=0)."""
    n = cap * 128
    buf = np.zeros(n, np.int16)
    buf[: len(idx_arr)] = idx_arr.astype(np.int16)
    blk = buf.reshape(n // 16, 16).T  # [16, n/16]
    return np.tile(blk, (8, 1))       # [128, n/16]


def _prep(edge_row, edge_col, edge_val):
    """Build per-core metadata + the static structure description."""
    owner = edge_row // SHARD
    per_core = []
    for c in range(NCORES):
        m = owner == c
        r = edge_row[m].astype(np.int64) - c * SHARD
        col = edge_col[m].astype(np.int64)
        val = edge_val[m]
        tile = r >> 7
        slot = r & 127
        win = col >> 15
        sg = tile // SG_TILES
        order = np.lexsort((tile, win, sg))
        per_core.append(dict(
            tile=tile[order], slot=slot[order], col=col[order],
            val=val[order], win=win[order], sg=sg[order]))

    # run partitions: key = sg*NWIN + win
    run_counts = np.zeros((NCORES, NSG * NWIN), np.int64)
    run_starts = np.zeros((NCORES, NSG * NWIN + 1), np.int64)
    for c in range(NCORES):
        d = per_core[c]
        key = d["sg"] * NWIN + d["win"]
        run_counts[c] = np.bincount(key, minlength=NSG * NWIN)
        run_starts[c, 1:] = np.cumsum(run_counts[c])

    cap_blk = np.zeros(NSG * NWIN, np.int64)
    for k in range(NSG * NWIN):
        cap_blk[k] = _ceil(int(run_counts[:, k].max()), 128)

    # per-sg gather-buffer block offsets (same layout every sg; sized by max)
    sg_bof = []
    sg_nblk = []
    for s in range(NSG):
        off = [0] * NWIN
        acc = 0
        for w in range(NWIN):
            off[w] = acc
            acc += int(cap_blk[s * NWIN + w])
        sg_bof.append(off)
        sg_nblk.append(acc)
    TOTBLK = max(sg_nblk)

    # block -> union of tiles (over cores); then tile-major MM slot list per sg
    mm_slots = []
    for s in range(NSG):
        tiles_here = list(range(s * SG_TILES, min((s + 1) * SG_TILES, NTILE)))
        cover = {}
        for w in range(NWIN):
            k = s * NWIN + w
            for b in range(int(cap_blk[k])):
                u = set()
                for c in range(NCORES):
                    st = run_starts[c, k]
                    n = run_counts[c, k]
                    lo = b * 128
                    hi = min(lo + 128, n)
                    if lo < n:
                        seg = per_core[c]["tile"][st + lo: st + hi]
                        u.update(np.unique(seg).tolist())
                cover[(w, b)] = u
        slots_s = []
        for t in tiles_here:
            for w in range(NWIN):
                for b in range(int(cap_blk[s * NWIN + w])):
                    if t in cover[(w, b)]:
                        slots_s.append((t - s * SG_TILES, w, b))
        mm_slots.append(slots_s)
    NMM = sum(len(x) for x in mm_slots)

    # per-core gidx + duplicated-pair scol2/sval2 (fp16)
    GCOLS = int(sum(cap_blk)) * 8
    gidx_arrs = []
    scol_arrs = []
    sval_arrs = []
    for c in range(NCORES):
        d = per_core[c]
        gidx = np.zeros((128, GCOLS), np.int16)
        scol2 = np.full((128, 2 * NMM), -1.0, np.float16)
        sval2 = np.zeros((128, 2 * NMM), np.float16)
        gcol_off = 0
        for s in range(NSG):
            for w in range(NWIN):
                k = s * NWIN + w
                cap = int(cap_blk[k])
                if cap == 0:
                    continue
                st, n = run_starts[c, k], run_counts[c, k]
                loc = d["col"][st: st + n] - w * WIN
                gidx[:, gcol_off: gcol_off + cap * 8] = _pack_idx(loc, cap)
                gcol_off += cap * 8
        mi = 0
        for s in range(NSG):
            for (tl, w, b) in mm_slots[s]:
                k = s * NWIN + w
                st, n = run_starts[c, k], run_counts[c, k]
                lo, hi = b * 128, min(b * 128 + 128, int(n))
                scol = np.full(128, -1.0, np.float16)
                vcol = np.zeros(128, np.float16)
                if lo < n:
                    seg_t = d["tile"][st + lo: st + hi]
                    seg_s = d["slot"][st + lo: st + hi]
                    seg_v = d["val"][st + lo: st + hi]
                    sel = seg_t == (s * SG_TILES + tl)
                    scol[: hi - lo][sel] = seg_s[sel]
                    vcol[: hi - lo][sel] = seg_v[sel].astype(np.float16)
                scol2[:, 2 * mi] = scol
                scol2[:, 2 * mi + 1] = scol
                sval2[:, 2 * mi] = vcol
                sval2[:, 2 * mi + 1] = vcol
                mi += 1
        gidx_arrs.append(gidx)
        scol_arrs.append(scol2)
        sval_arrs.append(sval2)

    structure = (
        tuple(cap_blk.tolist()),
        tuple(tuple(sl) for sg in mm_slots for sl in sg),
        tuple(len(sl) for sl in mm_slots),
        TOTBLK,
    )
    meta = dict(
        cap_blk=cap_blk, sg_bof=sg_bof, sg_nblk=sg_nblk, TOTBLK=TOTBLK,
        mm_slots=mm_slots, NMM=NMM, GCOLS=GCOLS,
        gidx_arrs=gidx_arrs, scol_arrs=scol_arrs, sval_arrs=sval_arrs,
    )
    return structure, meta


def _build_program(structure, meta):
    import concourse.bass as bass
    import concourse.mybir as mybir
    import concourse.tile as tile
    from concourse import bacc

    f16 = mybir.dt.float16
    f32 = mybir.dt.float32
    i16 = mybir.dt.int16
    AP = bass.AP

    cap_blk = meta["cap_blk"]
    sg_bof = meta["sg_bof"]
    mm_slots = meta["mm_slots"]
    NMM = meta["NMM"]
    GCOLS = meta["GCOLS"]
    TOTBLK = meta["TOTBLK"]

    nc = bacc.Bacc(None, num_devices=NCORES, num_swdge_queues=4)
    x0sh = nc.dram_tensor("x0sh", [SHARD, D], f32, kind="ExternalInput")
    x0f16 = nc.dram_tensor("x0f16", [NPAD, D], f16, kind="ExternalInput")
    gidx = nc.dram_tensor("gidx", [P, GCOLS], i16, kind="ExternalInput")
    scol2 = nc.dram_tensor("scol2", [P, 2 * NMM], f16, kind="ExternalInput")
    sval2 = nc.dram_tensor("sval2", [P, 2 * NMM], f16, kind="ExternalInput")
    w1 = nc.dram_tensor("w1", [D, D], f16, kind="ExternalInput")
    w2 = nc.dram_tensor("w2", [D, D], f16, kind="ExternalInput")
    brep1 = nc.dram_tensor("brep1", [P, D], f32, kind="ExternalInput")
    brep2 = nc.dram_tensor("brep2", [P, D], f32, kind="ExternalInput")
    xn_out = nc.dram_tensor("xn", [3, SHARD, D], f32, kind="ExternalOutput")

    # mm slot base offset per sg
    mi_base = [0] * NSG
    acc = 0
    for s in range(NSG):
        mi_base[s] = acc
        acc += len(mm_slots[s])
    # gidx col offsets per (s, w)
    gcol_off = [0] * (NSG * NWIN)
    acc = 0
    for s in range(NSG):
        for w in range(NWIN):
            gcol_off[s * NWIN + w] = acc
            acc += int(cap_blk[s * NWIN + w]) * 8

    with tile.TileContext(nc) as tc:
        with (
            tc.tile_pool(name="const", bufs=1) as cpool,
            tc.tile_pool(name="meta", bufs=1) as mpool,
            tc.tile_pool(name="gb", bufs=2) as gpool,
            tc.tile_pool(name="strip", bufs=3) as spool,
            tc.tile_pool(name="work", bufs=4) as wpool,
            tc.tile_pool(name="norm", bufs=2) as npool,
            tc.tile_pool(name="psy", bufs=4, space="PSUM") as psy,
            tc.tile_pool(name="psx", bufs=2, space="PSUM") as psx,
            tc.tile_pool(name="dram", bufs=1, space="DRAM") as dram,
        ):
            # constants
            iota_i = cpool.tile([P, P], mybir.dt.int32)
            nc.gpsimd.iota(iota_i, pattern=[[1, P]], base=0,
                           channel_multiplier=0)
            iota16 = cpool.tile([P, P], f16)
            nc.vector.tensor_copy(iota16, iota_i)
            eps_t = cpool.tile([P, 1], f32)
            nc.vector.memset(eps_t[:, :], 1e-24)
            w1_t = cpool.tile([P, P], f16)
            nc.sync.dma_start(out=w1_t, in_=w1[:, :])
            w2_t = cpool.tile([P, P], f16)
            nc.sync.dma_start(out=w2_t, in_=w2[:, :])
            b1_t = cpool.tile([P, P], f32)
            nc.sync.dma_start(out=b1_t, in_=brep1[:, :])
            b2_t = cpool.tile([P, P], f32)
            nc.sync.dma_start(out=b2_t, in_=brep2[:, :])
            gidx_t = mpool.tile([P, GCOLS], i16)
            nc.sync.dma_start(out=gidx_t, in_=gidx[:, :])
            scol_t = mpool.tile([P, 2 * NMM], f16)
            nc.sync.dma_start(out=scol_t, in_=scol2[:, :])
            sval_t = mpool.tile([P, 2 * NMM], f16)
            nc.sync.dma_start(out=sval_t, in_=sval2[:, :])

            # internal DRAM
            xsh1 = dram.tile([SHARD, D], f16)
            xg1 = dram.tile([NPAD, D], f16, addr_space="Shared")

            def strip_aps(strip, strip2, m0, S):
                """4-D packed-pair APs for the batched one-hot build."""
                st = strip[:, :, :]
                st4 = AP(st.tensor, st.offset,
                         [st.ap[0], [128, S], [2, 64], [1, 2]])
                st2 = strip2[:, :, :]
                st24 = AP(st2.tensor, st2.offset,
                          [st2.ap[0], [128, S], [2, 64], [1, 2]])
                io = iota16[:, :]
                io4 = AP(io.tensor, io.offset,
                         [io.ap[0], [0, S], [2, 64], [1, 2]])
                sc = scol_t[:, 2 * m0: 2 * (m0 + S)]
                sc4 = AP(sc.tensor, sc.offset,
                         [sc.ap[0], [2, S], [0, 64], [1, 2]])
                sv = sval_t[:, 2 * m0: 2 * (m0 + S)]
                sv4 = AP(sv.tensor, sv.offset,
                         [sv.ap[0], [2, S], [0, 64], [1, 2]])
                return st4, st24, io4, sc4, sv4

            def norm_rows(xin, ntl, dst_ap, rdt):
                """l2-normalize rows of xin [P, ntl, P] -> dst (f32)."""
                sq = npool.tile([P, SG_TILES, P], f32, tag="sq")
                nc.vector.tensor_tensor(
                    out=sq[:, :ntl, :], in0=xin[:, :ntl, :],
                    in1=xin[:, :ntl, :], op=mybir.AluOpType.mult)
                rs = npool.tile([P, SG_TILES], f32, tag="rs")
                nc.vector.tensor_reduce(
                    out=rs[:, :ntl], in_=sq[:, :ntl, :],
                    axis=mybir.AxisListType.X, op=mybir.AluOpType.add)
                nrm = npool.tile([P, SG_TILES], f32, tag="nrm")
                nc.scalar.activation(nrm[:, :ntl], rs[:, :ntl],
                                     mybir.ActivationFunctionType.Sqrt,
                                     bias=eps_t[:, :1])
                rinv = npool.tile([P, SG_TILES], rdt, tag="rinv")
                with nc.allow_low_precision(reason="f16 rinv; 5e-4 ok"):
                    nc.vector.reciprocal(rinv[:, :ntl], nrm[:, :ntl])
                xo = npool.tile([P, SG_TILES, P], f32, tag="xo")
                ri = rinv[:, :ntl]
                ri_b = AP(ri.tensor, ri.offset, [ri.ap[0], [1, ntl], [0, P]])
                nc.vector.tensor_tensor(
                    out=xo[:, :ntl, :], in0=xin[:, :ntl, :], in1=ri_b,
                    op=mybir.AluOpType.mult)
                nc.sync.dma_start(
                    out=dst_ap.rearrange("(c p) d -> p c d", p=P),
                    in_=xo[:, :ntl, :])

            def pair0_stage():
                """hop-0: l2norm own shard rows from exact f32 input."""
                for s in range(NSG):
                    ntl = min(SG_TILES, NTILE - s * SG_TILES)
                    r0 = s * SG_TILES * 128
                    x0t = npool.tile([P, SG_TILES, P], f32, tag="x0t")
                    nc.sync.dma_start(
                        out=x0t[:, :ntl, :],
                        in_=x0sh[r0: r0 + ntl * 128, :].rearrange(
                            "(c p) d -> p c d", p=P))
                    norm_rows(x0t, ntl, xn_out[0, r0: r0 + ntl * 128, :],
                              f32)

            def graph_hop(src, w_t, b_t, xsh, hop):
                """One GCN hop: x_new = (A @ src) @ W + b; write l2norm of
                own-shard rows to xn_out[hop]; optionally publish xsh."""
                qload = [0, 0, 0, 0]
                for s in range(NSG):
                    gbuf = gpool.tile([P, TOTBLK, P], f16, tag="gbuf")
                    CH = 11
                    for w in range(NWIN):
                        k = s * NWIN + w
                        cap = int(cap_blk[k])
                        if cap == 0:
                            continue
                        hi = min(NPAD, (w + 1) * WIN)
                        for lo in range(0, cap, CH):
                            ln = min(CH, cap - lo)
                            q = qload.index(min(qload))
                            qload[q] += ln
                            bo = sg_bof[s][w] + lo
                            co = gcol_off[k] + lo * 8
                            nc.gpsimd.dma_gather(
                                gbuf[:, bo: bo + ln, :],
                                src[w * WIN: hi, :],
                                gidx_t[:, co: co + ln * 8],
                                num_idxs=ln * 128, num_idxs_reg=ln * 128,
                                elem_size=P, single_packet=False,
                                queue_num=q,
                            )
                    slots = mm_slots[s]
                    ntl = min(SG_TILES, NTILE - s * SG_TILES)
                    xrows = npool.tile([P, SG_TILES, P], f16, tag="xrows")
                    for t in range(ntl):
                        tslots = [(i, sl) for i, sl in enumerate(slots)
                                  if sl[0] == t]
                        S = len(tslots)
                        m0 = mi_base[s] + tslots[0][0]
                        strip = spool.tile([P, S, P], f16, tag="strip")
                        strip2 = spool.tile([P, S, P], f16, tag="strip2")
                        st4, st24, io4, sc4, sv4 = strip_aps(
                            strip, strip2, m0, S)
                        nc.vector.tensor_tensor(
                            out=st4, in0=io4, in1=sc4,
                            op=mybir.AluOpType.is_equal)
                        nc.vector.tensor_tensor(
                            out=st24, in0=st4, in1=sv4,
                            op=mybir.AluOpType.mult)
                        y_ps = psy.tile([P, P], f32, space="PSUM", tag="y")
                        for si, (i, (tl, w, b)) in enumerate(tslots):
                            gb = sg_bof[s][w] + b
                            nc.tensor.matmul(
                                y_ps, lhsT=gbuf[:, gb, :],
                                rhs=strip2[:, si, :],
                                start=(si == 0), stop=(si == S - 1),
                            )
                        yT = wpool.tile([P, P], f16, tag="yT")
                        nc.scalar.copy(yT, y_ps)
                        x_ps = psx.tile([P, P], f32, space="PSUM", tag="x")
                        nc.tensor.matmul(x_ps, lhsT=yT, rhs=w_t,
                                         start=True, stop=True)
                        nc.vector.tensor_tensor(
                            out=xrows[:, t, :], in0=x_ps[:, :], in1=b_t[:, :],
                            op=mybir.AluOpType.add)
                    r0 = s * SG_TILES * 128
                    if xsh is not None:
                        nc.sync.dma_start(
                            out=xsh[r0: r0 + ntl * 128, :].rearrange(
                                "(c p) d -> p c d", p=P),
                            in_=xrows[:, :ntl, :])
                    norm_rows(xrows, ntl,
                              xn_out[hop, r0: r0 + ntl * 128, :], f16)

            stages = os.environ.get(
                "BASS_GNN_STAGES", "p0,h1,ag1,h2").split(",")
            if "p0" in stages:
                pair0_stage()
            if "h1" in stages:
                graph_hop(x0f16, w1_t, b1_t, xsh1, 1)
            if "ag1" in stages:
                nc.gpsimd.collective_compute(
                    "AllGather", mybir.AluOpType.bypass,
                    replica_groups=[list(range(NCORES))],
                    ins=[xsh1.opt()], outs=[xg1.opt()],
                )
            if "h2" in stages:
                graph_hop(xg1, w2_t, b2_t, None, 2)

    nc.compile()
    return nc


def _install_ntff_shim():
    """Provide antenv.axon_hooks (missing on this image) so trace=True can
    capture NTFF profiles through the axon .so."""
    import types
    if "antenv.axon_hooks" in sys.modules:
        return
    mod = types.ModuleType("antenv.axon_hooks")
    mod._hook = None

    def set_axon_ntff_profile_hook(h):
        mod._hook = h

    def get_axon_ntff_profile_hook():
        return mod._hook

    mod.set_axon_ntff_profile_hook = set_axon_ntff_profile_hook
    mod.get_axon_ntff_profile_hook = get_axon_ntff_profile_hook
    sys.modules["antenv.axon_hooks"] = mod
    try:
        from trn_agent_boot.trn_boot import _ntff_profile_via_ctypes
        mod._hook = _ntff_profile_via_ctypes("/opt/axon/libaxon_pjrt.so")
    except Exception:
        mod._hook = None


def kernel(node_emb, attri_emb, W1, b1, W2, b2, edge_val,
           edge_row, edge_col, pos_src, pos_dst, neg_src, neg_dst):
    global LAST_RESULTS
    _install_ntff_shim()
    from concourse.bass_utils import run_bass_kernel_spmd

    structure, meta = _prep(edge_row, edge_col, edge_val)

    import time as _time
    key = (structure, os.environ.get("BASS_GNN_STAGES", ""))
    if key in _CACHE:
        nc = _CACHE[key]
    else:
        t0 = _time.time()
        nc = _build_program(structure, meta)
        print(f"[kernel] build+schedule: {_time.time() - t0:.1f}s, "
              f"{len(nc.inst_map)} instructions", flush=True)
        _CACHE[key] = nc

    x0 = np.concatenate([node_emb, attri_emb], axis=0).astype(np.float32)
    x0p = np.zeros((NPAD, D), np.float32)
    x0p[:N] = x0
    x0p16 = x0p.astype(np.float16)

    in_maps = []
    for c in range(NCORES):
        in_maps.append({
            "x0sh": x0p[c * SHARD: (c + 1) * SHARD],
            "x0f16": x0p16,
            "gidx": meta["gidx_arrs"][c],
            "scol2": meta["scol_arrs"][c],
            "sval2": meta["sval_arrs"][c],
            "w1": W1.astype(np.float16),
            "w2": W2.astype(np.float16),
            "brep1": np.broadcast_to(
                b1.astype(np.float32)[None, :], (P, D)).copy(),
            "brep2": np.broadcast_to(
                b2.astype(np.float32)[None, :], (P, D)).copy(),
        })

    trace = os.environ.get("BASS_GNN_TRACE", "0") == "1"
    t0 = _time.time()
    res = run_bass_kernel_spmd(nc, in_maps, core_ids=list(range(NCORES)),
                               trace=trace)
    print(f"[kernel] compile+run: {_time.time() - t0:.1f}s", flush=True)
    LAST_RESULTS = res

    # ---- host assembly: index normalized tables per hop ----
    xn_full = np.empty((3, NPAD, D), np.float32)
    for c in range(NCORES):
        xn_full[:, c * SHARD: (c + 1) * SHARD] = res.results[c]["xn"]
    out = np.empty((4, 3, E_PAIR, D), np.float32)
    for st, idx in enumerate((pos_src, pos_dst, neg_src, neg_dst)):
        idx64 = idx.astype(np.int64)
        for h in range(3):
            out[st, h] = xn_full[h, idx64]
    return out
